# revision 37
# baseline (speedup 1.0000x reference)
"""Trainium2 Bass kernel for nn_HeatmapEncoder.

Math per (b, s, c):
    g_t = exp(-((gx-cx_t)^2 + (gy-cy_t)^2) / (2 sigma^2)),  t in {gaze, hand}
    g_t = g_t / (sum(g_t) + eps)        (zeroed when cx+cy <= 0)
    unified = g_gaze + g_hand
    out = unified / (max(unified) + eps)

Each normalized Gaussian is separable, so unified is rank-2 and each map
is generated by ONE K=2 bf16 matmul per 112-row chunk (y factors carry
the per-set sum-normalization amplitude a).  The peak max(unified) is
NOT computed from the generated map: all critical points of a sum of two
isotropic Gaussians lie on the line through the two centers, so the peak
is evaluated at 16 candidate offsets (sigma/8 apart) from each center
toward the other, needing only the 1-D factor sums and the center
distance (host-precomputed).  Peak error ~0.25%, bf16 factors ~0.4% --
well inside the 2e-2 gate.

All 256 1-D factors (x and y of both sets) are produced by a single
Square+Exp ACT pass on a [128, 336] tile (x factors rows 0..63, y rows
64..127).  Cross-partition moves stay on-chip via tiny PE matmuls: a
[128,128] half-swap permutation pairs Sx with Sy, a [64,64] permutation
swaps partner amplitudes and pair values (max(a,b) = (a+b+|a-b|)/2), and
a transpose + ones-broadcast turns the [32] per-map reciprocals into the
[112, 32] scale tile the drains read.  The PSUM drain is one fused
scale+cast pass per PSUM region (c0,c1 share a 2-bank tile), and bf16
output DMA groups of 4 maps alternate between two queues (host upcasts).

Layout: factor rows are t-major (gaze rows 0..31, hand rows 32..63).
Map j = 4b + q keeps its factor pair at SBUF partitions 32q, 32q+1, free
block b (PE row tiles are 32-aligned; cycling q hides LDWEIGHTS).  Map
rows are interleaved y = 3p + c so each map is one contiguous DRAM range.

Sharding: pure data parallel over batch B=8 across the 8 cores.
"""

import functools
from contextlib import ExitStack

import numpy as np

try:
    import concourse.bass as bass
except ImportError:  # pragma: no cover
    import sys

    sys.path.insert(0, "/opt/trn_rl_repo")
    import concourse.bass as bass

import concourse.tile as tile
from concourse import bacc, mybir
from concourse.bass_utils import run_bass_kernel_spmd

H = W = 336
P = 112  # partitions per y-chunk; y = 3*p + c  (c in 0..2)
NCH = 3
S_DIM, C_DIM = 8, 4
NMAPS = S_DIM * C_DIM  # 32 maps per core
NR = 2 * NMAPS  # 64 factor rows per axis, t-major: row = 32*t + j
NF = 2 * NR  # 128 fused factor rows: x rows 0..63, y rows 64..127
N_CORES = 8
SIGMA = 10.0 / 336.0
EXP_SCALE = -1.0 / (2.0 * SIGMA * SIGMA)
EPS = 1e-6
NCAND = 16  # candidate peak offsets k*sigma/8 toward the partner center
CSTEP = SIGMA / 8.0
F32 = mybir.dt.float32
BF16 = mybir.dt.bfloat16
AF = mybir.ActivationFunctionType
ALU = mybir.AluOpType
AX = mybir.AxisListType


def _emit(nc, tc, ctx, negcd_in, out_t, aux_in, stg):
    const = ctx.enter_context(tc.tile_pool(name="const", bufs=1))
    fact = ctx.enter_context(tc.tile_pool(name="fact", bufs=1))
    ffac = ctx.enter_context(tc.tile_pool(name="ffac", bufs=1))
    small = ctx.enter_context(tc.tile_pool(name="small", bufs=2))
    sstage = ctx.enter_context(tc.tile_pool(name="sstage", bufs=3))
    # per map: all 3 chunks packed tight into TWO banks (c1's matmul is
    # split at the bank boundary), giving 4 maps of PSUM depth and ONE
    # contiguous fused drain per map (fewest semaphore items; the
    # sequencers, not the engines, pace the steady state).  The head's
    # PE-trick outputs borrow rotation slots
    pone = ctx.enter_context(tc.tile_pool(name="pone", bufs=4, space="PSUM"))

    # ---- inputs: one first DMA per queue (first-DMA latency ~10us
    # gates the head); ACT table preload via dummy exp on a DVE memset ----
    dum = small.tile([1, 16], F32, tag="dum")
    nc.gpsimd.memset(dum[:], 0.0)
    dum2 = small.tile([1, 16], F32, tag="dum2")
    nc.scalar.activation(dum2[:], dum[:], AF.Exp, bias=0.0, scale=1.0)
    NCDC = const.tile([NF, 3 + W + 2 * NCAND], F32)
    nc.sync.dma_start(NCDC[:], negcd_in.ap())
    AUX = const.tile([NF, NR + NF + NMAPS + P], F32)
    nc.gpsimd.dma_start(AUX[:], aux_in.ap())
    NCD = NCDC[:, 0:3]
    G = NCDC[:, 3:3 + W]
    E1c = NCDC[0:NR, 3 + W:3 + W + NCAND]
    TKc = NCDC[0:NR, 3 + W + NCAND:3 + W + 2 * NCAND]
    P64c = AUX[0:NR, 0:NR]
    P128c = AUX[:, NR:NR + NF]
    ID32c = AUX[0:NMAPS, NR + NF:NR + NF + NMAPS]
    onesC = AUX[0:1, NR + NF:NR + NF + NMAPS + P][:, NMAPS:]

    # ---- all 1-D gaussian factors in one [128, 336] pass, bf16 out ----
    sq = fact.tile([NF, W], F32)
    nc.scalar.activation(sq[:], G, AF.Square, bias=NCD[:, 0:1], scale=1.0)
    fb = fact.tile([NF, W], BF16)
    nc.scalar.activation(fb[:], sq[:], AF.Exp, bias=0.0, scale=EXP_SCALE)
    nc.sync.dma_start(stg.ap()[0], fb[0:NR, :])  # x side stages early
    sxy = small.tile([NF, 1], F32, tag="sxy")
    nc.vector.reduce_sum(sxy[:], fb[:], axis=AX.X)

    # candidate partner-distance exponentials (ACT, off critical path)
    sq2 = small.tile([NR, NCAND], F32, tag="sq2")
    nc.scalar.activation(sq2[:], TKc, AF.Square, bias=NCD[0:NR, 2:3],
                         scale=1.0)
    e2 = small.tile([NR, NCAND], F32, tag="e2")
    nc.scalar.activation(e2[:], sq2[:], AF.Exp, bias=0.0, scale=EXP_SCALE)

    # amplitude a = valid / (Sx*Sy + eps); Sx meets Sy via PE half-swap
    sswP = pone.tile([NF, 256], F32, tag="po")
    nc.tensor.matmul(sswP[:, 0:1], P128c, sxy[:], start=True, stop=True,
                     tile_position=(0, 0))
    ss = small.tile([NF, 1], F32, tag="ss")
    nc.vector.tensor_mul(ss[:], sxy[:], sswP[:, 0:1])
    sse = small.tile([NF, 1], F32, tag="sse")
    nc.vector.tensor_scalar_add(sse[:], ss[:], EPS)
    rec = small.tile([NF, 1], F32, tag="rec")
    nc.vector.reciprocal(rec[:], sse[:])
    vs = small.tile([NF, 1], F32, tag="vs")
    nc.vector.tensor_add(vs[:], NCD[:, 0:1], NCD[:, 1:2])
    vm = small.tile([NF, 1], F32, tag="vm")  # valid: (-cx)+(-cy) < 0
    nc.vector.tensor_scalar(vm[:], vs[:], 0.0, None, op0=ALU.is_lt)
    av = small.tile([NF, 1], F32, tag="av")
    nc.vector.tensor_mul(av[:], rec[:], vm[:])
    ybs = fact.tile([NF, W], BF16)
    nc.vector.tensor_scalar_mul(ybs[NR:NF, :], fb[NR:NF, :],
                                av[NR:NF, 0:1])
    nc.gpsimd.dma_start(stg.ap()[1], ybs[NR:NF, :])

    # ---- scatter factor pairs into the 32-aligned K=2 layout ----
    # dest partitions 32q+u, free block b  <-  stage row 32u + 4b + q
    FXT = ffac.tile([128, S_DIM, W], BF16, name="FXT", tag="fxt")
    FYT = ffac.tile([128, S_DIM, W], BF16, name="FYT", tag="fyt")
    for u in range(2):
        src_x = stg.ap()[0].rearrange("(u b q) x -> u q b x", u=2, q=4)[u]
        src_y = stg.ap()[1].rearrange("(u b q) x -> u q b x", u=2, q=4)[u]
        dst_x = FXT[:].rearrange("(q u) b x -> u q b x", q=4)[u]
        dst_y = FYT[:].rearrange("(q u) b x -> u q b x", q=4)[u]
        eng = nc.sync if u == 0 else nc.gpsimd
        eng.dma_start(dst_x, src_x)
        eng.dma_start(dst_y, src_y)

    # ---- candidate peak: u(t_k) = a*E1[k] + a_partner*exp(S*(t_k-d)^2).
    # Partner amplitudes via PE permutation matmul (32-row pair swap);
    # pair-max via max(a,b) = (a+b+|a-b|)/2 on a second permuted matmul ----
    avpP = pone.tile([NR, 256], F32, tag="po")
    nc.tensor.matmul(avpP[:, 0:1], P64c, av[0:NR, :], start=True, stop=True,
                     tile_position=(0, 0))
    uu = small.tile([NR, NCAND], F32, tag="uu")
    nc.vector.tensor_scalar_mul(uu[:], E1c, av[0:NR, 0:1])
    up = small.tile([NR, NCAND], F32, tag="up")
    nc.vector.tensor_scalar_mul(up[:], e2[:], avpP[:, 0:1])
    ub = small.tile([NR, NCAND], F32, tag="ub")
    nc.vector.tensor_add(ub[:], uu[:], up[:])
    pm = small.tile([NR, 1], F32, tag="pm")
    nc.vector.reduce_max(pm[:], ub[:], axis=AX.X)
    pswP = pone.tile([NR, 256], F32, tag="po")
    nc.tensor.matmul(pswP[:, 0:1], P64c, pm[:], start=True, stop=True,
                     tile_position=(0, 0))
    sm = small.tile([NR, 1], F32, tag="sm")
    nc.vector.tensor_add(sm[:], pm[:], pswP[:, 0:1])
    df = small.tile([NR, 1], F32, tag="df")
    nc.vector.tensor_sub(df[:], pm[:], pswP[:, 0:1])
    ng = small.tile([NR, 1], F32, tag="ng")
    nc.vector.tensor_scalar_mul(ng[:], df[:], -1.0)
    ad = small.tile([NR, 1], F32, tag="ad")
    nc.vector.tensor_max(ad[:], df[:], ng[:])
    mx = small.tile([NR, 1], F32, tag="mx")
    nc.vector.tensor_add(mx[:], sm[:], ad[:])
    pke = small.tile([NR, 1], F32, tag="pke")  # 0.5*(s+|d|) + eps
    nc.vector.tensor_scalar(pke[:], mx[:], 0.5, EPS, op0=ALU.mult,
                            op1=ALU.add)
    rg = small.tile([NR, 1], F32, tag="rg")
    nc.vector.reciprocal(rg[:], pke[:])
    # transpose rows 0..31 into a free-dim row, then ones-broadcast to [112]
    rgTP = pone.tile([1, 256], F32, tag="po")
    nc.tensor.matmul(rgTP[:, 0:NMAPS], rg[0:NMAPS, 0:1], ID32c, start=True,
                     stop=True, is_transpose=True, tile_position=(0, 0))
    rgT = small.tile([1, NMAPS], F32, tag="rgT")
    nc.vector.tensor_copy(rgT[:], rgTP[:, 0:NMAPS])
    rgBP = pone.tile([P, 512], F32, tag="po")
    nc.tensor.matmul(rgBP[:, 0:NMAPS], onesC, rgT[:], start=True, stop=True,
                     tile_position=(0, 0))
    rgB = const.tile([P, NMAPS], F32)
    nc.vector.tensor_copy(rgB[:], rgBP[:, 0:NMAPS])

    # DRAM view matching stage layout: out[m, y, x], y = 3p+c, z = 336c+x
    dview = out_t.ap().rearrange("m (p c) x -> p m (c x)", p=P)

    def drain(eng, sview, pview, j):
        if eng == "scalar":
            nc.scalar.mul(sview, pview, rgB[:, j:j + 1])
        else:
            nc.vector.tensor_scalar_mul(sview, pview, rgB[:, j:j + 1])

    bacc_ = 0
    groups = [4, 4, 4, 4, 4, 4, 4, 2, 2]
    j0 = 0
    for gi, gsz in enumerate(groups):
        st = sstage.tile([P, 4, NCH * W], BF16, tag="sst")
        for j in range(j0, j0 + gsz):
            q, b = j % 4, j // 4
            rhs = FXT[32 * q:32 * q + 2, b, :]
            lhsT = [FYT[32 * q:32 * q + 2, b, cix::3] for cix in range(NCH)]
            pt = pone.tile([P, 1024], F32, tag="po")
            # chunk c1 is split at the col-512 bank boundary (psum matmul
            # dests must stay inside one 2KB bank)
            nc.tensor.matmul(pt[:, 0:W], lhsT[0], rhs, start=True,
                             stop=True, tile_position=(32 * q, 0))
            nc.tensor.matmul(pt[:, W:512], lhsT[1], rhs[:, 0:512 - W],
                             start=True, stop=True,
                             tile_position=(32 * q, 0))
            nc.tensor.matmul(pt[:, 512:2 * W], lhsT[1], rhs[:, 512 - W:],
                             start=True, stop=True,
                             tile_position=(32 * q, 0))
            nc.tensor.matmul(pt[:, 2 * W:3 * W], lhsT[2], rhs, start=True,
                             stop=True, tile_position=(32 * q, 0))
            stj = st[:, j - j0, :]
            bacc_ += 17  # 17 of 32 map drains on ACT (ACT is faster)
            eng = "scalar" if bacc_ >= 32 else "vector"
            if bacc_ >= 32:
                bacc_ -= 32
            drain(eng, stj, pt[:, 0:NCH * W], j)
        eng = nc.sync if gi % 2 == 0 else nc.gpsimd
        eng.dma_start(dview[:, j0:j0 + gsz, :], st[:, 0:gsz, :])
        j0 += gsz


@functools.lru_cache(maxsize=1)
def _build():
    nc = bacc.Bacc("TRN2", target_bir_lowering=False, debug=False)
    negcd_in = nc.dram_tensor("negcd", [NF, 3 + W + 2 * NCAND], F32,
                              kind="ExternalInput")
    out_t = nc.dram_tensor("out", [NMAPS, H, W], BF16, kind="ExternalOutput")

    aux = np.zeros((NF, NR + NF + NMAPS + P), dtype=np.float32)
    for k in range(NR):  # P64: out[m] = in[(m+32)%64]
        aux[k, (k + NMAPS) % NR] = 1.0
    for k in range(NF):  # P128: out[m] = in[(m+64)%128]
        aux[k, NR + (k + NR) % NF] = 1.0
    aux[0:NMAPS, NR + NF:NR + NF + NMAPS] = np.eye(NMAPS, dtype=np.float32)
    aux[0, NR + NF + NMAPS:] = 1.0
    aux_in = nc.inline_tensor(aux, name="auxc")

    stg = nc.dram_tensor("stg", [2, NR, W], BF16)

    with tile.TileContext(nc) as tc, ExitStack() as ctx:
        _emit(nc, tc, ctx, negcd_in, out_t, aux_in, stg)
    nc.compile()
    return nc


_grid = (np.arange(W, dtype=np.float64) / (W - 1)).astype(np.float32)
_tk = (np.arange(NCAND, dtype=np.float64) * CSTEP).astype(np.float32)
_e1 = np.exp(-(_tk.astype(np.float64) ** 2) / (2.0 * SIGMA ** 2)).astype(
    np.float32)
_ROWC = np.tile(np.concatenate([_grid, _e1, _tk]), (NF, 1))


def _in_map_for(gaze, hand, b):
    cg = np.asarray(gaze[b], dtype=np.float32).reshape(NMAPS, 2)
    ch = np.asarray(hand[b], dtype=np.float32).reshape(NMAPS, 2)
    d = np.sqrt(((cg - ch) ** 2).sum(axis=1))
    cxs = np.concatenate([cg[:, 0], ch[:, 0]])  # t-major per axis
    cys = np.concatenate([cg[:, 1], ch[:, 1]])
    dd = np.concatenate([d, d])
    # rows 0..63: x factors (-cx, -cy, -d); rows 64..127: y (-cy, -cx, -d);
    # then the row-constants [grid | E1 | tk] are packed alongside so one
    # DMA carries every per-row operand
    top = np.stack([-cxs, -cys, -dd], axis=1)
    bot = np.stack([-cys, -cxs, -dd], axis=1)
    ncd = np.concatenate([top, bot], axis=0)
    return {"negcd": np.ascontiguousarray(np.concatenate(
        [ncd, _ROWC], axis=1).astype(np.float32))}


def kernel(gaze_coords, hand_coords, _trace=False, **trace_kwargs):
    gaze_coords = np.asarray(gaze_coords, dtype=np.float32)
    hand_coords = np.asarray(hand_coords, dtype=np.float32)
    B = gaze_coords.shape[0]
    assert B == N_CORES, f"expected batch {N_CORES}, got {B}"
    nc = _build()
    in_maps = [_in_map_for(gaze_coords, hand_coords, b) for b in range(B)]
    res = run_bass_kernel_spmd(nc, in_maps, list(range(N_CORES)),
                               trace=_trace, **trace_kwargs)
    out = np.stack(
        [np.asarray(res.results[i]["out"]).astype(np.float32).reshape(
            S_DIM, C_DIM, H, W) for i in range(B)],
        axis=0,
    )
    if _trace:
        return out, res
    return out


# revision 42
# speedup vs baseline: 1.1486x; 1.1486x over previous
"""Trainium2 Bass kernel for nn_HeatmapEncoder.

Math per (b, s, c):
    g_t = exp(-((gx-cx_t)^2 + (gy-cy_t)^2) / (2 sigma^2)),  t in {gaze, hand}
    g_t = g_t / (sum(g_t) + eps)        (zeroed when cx+cy <= 0)
    unified = g_gaze + g_hand
    out = unified / (max(unified) + eps)

Each normalized Gaussian is separable, so unified is rank-2 and each map
is generated by ONE K=2 bf16 matmul per 112-row chunk (y factors carry
the per-set sum-normalization amplitude a).  The peak max(unified) is
NOT computed from the generated map: all critical points of a sum of two
isotropic Gaussians lie on the line through the two centers, so the peak
is evaluated at 16 candidate offsets (sigma/8 apart) from each center
toward the other, needing only the 1-D factor sums and the center
distance (host-precomputed).  Peak error ~0.25%, bf16 factors ~0.4% --
well inside the 2e-2 gate.

All 256 1-D factors (x and y of both sets) are produced by a single
Square+Exp ACT pass on a [128, 336] tile (x factors rows 0..63, y rows
64..127).  Cross-partition moves stay on-chip via tiny PE matmuls: a
[128,128] half-swap permutation pairs Sx with Sy, a [64,64] permutation
swaps partner amplitudes and pair values (max(a,b) = (a+b+|a-b|)/2), and
a transpose + ones-broadcast turns the [32] per-map reciprocals into the
[112, 32] scale tile the drains read.  The PSUM drain is one fused
scale+cast pass per map (each map packed tight into 2 PSUM banks by
splitting chunk c1's matmul at the bank boundary, so 4 maps are in
flight across the 8 banks), and bf16 output DMA groups of 4 maps
alternate between two queues (host upcasts to fp32).

Layout: factor rows are t-major (gaze rows 0..31, hand rows 32..63).
Map j = 4b + q keeps its factor pair at SBUF partitions 32q, 32q+1, free
block b (PE row tiles are 32-aligned; cycling q hides LDWEIGHTS).  Map
rows are interleaved y = 3p + c so each map is one contiguous DRAM range.

Sharding: pure data parallel over batch B=8 across the 8 cores.
"""

import functools
from contextlib import ExitStack

import numpy as np

try:
    import concourse.bass as bass
except ImportError:  # pragma: no cover
    import sys

    sys.path.insert(0, "/opt/trn_rl_repo")
    import concourse.bass as bass

import concourse.tile as tile
from concourse import bacc, mybir
from concourse.bass_utils import run_bass_kernel_spmd

H = W = 336
P = 112  # partitions per y-chunk; y = 3*p + c  (c in 0..2)
NCH = 3
S_DIM, C_DIM = 8, 4
NMAPS = S_DIM * C_DIM  # 32 maps per core
NR = 2 * NMAPS  # 64 factor rows per axis, t-major: row = 32*t + j
NF = 2 * NR  # 128 fused factor rows: x rows 0..63, y rows 64..127
N_CORES = 8
SIGMA = 10.0 / 336.0
EXP_SCALE = -1.0 / (2.0 * SIGMA * SIGMA)
EPS = 1e-6
NCAND = 16  # candidate peak offsets k*sigma/8 toward the partner center
CSTEP = SIGMA / 8.0
F32 = mybir.dt.float32
BF16 = mybir.dt.bfloat16
AF = mybir.ActivationFunctionType
ALU = mybir.AluOpType
AX = mybir.AxisListType


def _emit(nc, tc, ctx, negcd_in, out_t, aux_in, stg):
    const = ctx.enter_context(tc.tile_pool(name="const", bufs=1))
    fact = ctx.enter_context(tc.tile_pool(name="fact", bufs=1))
    ffac = ctx.enter_context(tc.tile_pool(name="ffac", bufs=1))
    small = ctx.enter_context(tc.tile_pool(name="small", bufs=1))
    sstage = ctx.enter_context(tc.tile_pool(name="sstage", bufs=3))
    # per map: all 3 chunks packed tight into TWO banks (c1's matmul is
    # split at the bank boundary), giving 4 maps of PSUM depth and ONE
    # contiguous fused drain per map (fewest semaphore items; the
    # sequencers, not the engines, pace the steady state).  The head's
    # PE-trick outputs borrow rotation slots
    pone = ctx.enter_context(tc.tile_pool(name="pone", bufs=4, space="PSUM"))

    # ---- inputs: one first DMA per queue (first-DMA latency ~10us
    # gates the head); ACT table preload via dummy exp on a DVE memset ----
    dum = small.tile([1, 16], F32, tag="dum")
    nc.gpsimd.memset(dum[:], 0.0)
    dum2 = small.tile([1, 16], F32, tag="dum2")
    nc.scalar.activation(dum2[:], dum[:], AF.Exp, bias=0.0, scale=1.0)
    NCDC = const.tile([NF, 3 + W + 2 * NCAND], F32)
    nc.sync.dma_start(NCDC[:], negcd_in.ap())
    AUX = const.tile([NF, NR + NF + NMAPS + P], F32)
    nc.gpsimd.dma_start(AUX[:], aux_in.ap())
    NCD = NCDC[:, 0:3]
    G = NCDC[:, 3:3 + W]
    E1c = NCDC[0:NR, 3 + W:3 + W + NCAND]
    TKc = NCDC[0:NR, 3 + W + NCAND:3 + W + 2 * NCAND]
    P64c = AUX[0:NR, 0:NR]
    P128c = AUX[:, NR:NR + NF]
    ID32c = AUX[0:NMAPS, NR + NF:NR + NF + NMAPS]
    onesC = AUX[0:1, NR + NF:NR + NF + NMAPS + P][:, NMAPS:]

    # ---- all 1-D gaussian factors in one [128, 336] pass, bf16 out ----
    sq = fact.tile([NF, W], F32)
    nc.scalar.activation(sq[:], G, AF.Square, bias=NCD[:, 0:1], scale=1.0)
    fb = fact.tile([NF, W], BF16)
    nc.scalar.activation(fb[:], sq[:], AF.Exp, bias=0.0, scale=EXP_SCALE)
    nc.sync.dma_start(stg.ap()[0], fb[0:NR, :])  # x side stages early
    sxy = small.tile([NF, 1], F32, tag="sxy")
    nc.vector.reduce_sum(sxy[:], fb[:], axis=AX.X)

    # candidate partner-distance exponentials (ACT, off critical path)
    sq2 = small.tile([NR, NCAND], F32, tag="sq2")
    nc.scalar.activation(sq2[:], TKc, AF.Square, bias=NCD[0:NR, 2:3],
                         scale=1.0)
    e2 = small.tile([NR, NCAND], F32, tag="e2")
    nc.scalar.activation(e2[:], sq2[:], AF.Exp, bias=0.0, scale=EXP_SCALE)

    # amplitude a = valid / (Sx*Sy + eps); Sx meets Sy via PE half-swap
    sswP = pone.tile([NF, 256], F32, tag="po")
    nc.tensor.matmul(sswP[:, 0:1], P128c, sxy[:], start=True, stop=True,
                     tile_position=(0, 0))
    ss = small.tile([NF, 1], F32, tag="ss")
    nc.vector.tensor_mul(ss[:], sxy[:], sswP[:, 0:1])
    sse = small.tile([NF, 1], F32, tag="sse")
    nc.vector.tensor_scalar_add(sse[:], ss[:], EPS)
    rec = small.tile([NF, 1], F32, tag="rec")
    nc.vector.reciprocal(rec[:], sse[:])
    vs = small.tile([NF, 1], F32, tag="vs")
    nc.vector.tensor_add(vs[:], NCD[:, 0:1], NCD[:, 1:2])
    vm = small.tile([NF, 1], F32, tag="vm")  # valid: (-cx)+(-cy) < 0
    nc.vector.tensor_scalar(vm[:], vs[:], 0.0, None, op0=ALU.is_lt)
    av = small.tile([NF, 1], F32, tag="av")
    nc.vector.tensor_mul(av[:], rec[:], vm[:])
    ybs = fact.tile([NF, W], BF16)
    nc.vector.tensor_scalar_mul(ybs[NR:NF, :], fb[NR:NF, :],
                                av[NR:NF, 0:1])
    nc.gpsimd.dma_start(stg.ap()[1], ybs[NR:NF, :])

    # ---- scatter factor pairs into the 32-aligned K=2 layout ----
    # dest partitions 32q+u, free block b  <-  stage row 32u + 4b + q
    FXT = ffac.tile([128, S_DIM, W], BF16, name="FXT", tag="fxt")
    FYT = ffac.tile([128, S_DIM, W], BF16, name="FYT", tag="fyt")
    for u in range(2):
        src_x = stg.ap()[0].rearrange("(u b q) x -> u q b x", u=2, q=4)[u]
        src_y = stg.ap()[1].rearrange("(u b q) x -> u q b x", u=2, q=4)[u]
        dst_x = FXT[:].rearrange("(q u) b x -> u q b x", q=4)[u]
        dst_y = FYT[:].rearrange("(q u) b x -> u q b x", q=4)[u]
        eng = nc.sync if u == 0 else nc.gpsimd
        eng.dma_start(dst_x, src_x)
        eng.dma_start(dst_y, src_y)

    # ---- candidate peak: u(t_k) = a*E1[k] + a_partner*exp(S*(t_k-d)^2).
    # Partner amplitudes via PE permutation matmul (32-row pair swap);
    # pair-max via max(a,b) = (a+b+|a-b|)/2 on a second permuted matmul ----
    avpP = pone.tile([NR, 256], F32, tag="po")
    nc.tensor.matmul(avpP[:, 0:1], P64c, av[0:NR, :], start=True, stop=True,
                     tile_position=(0, 0))
    uu = small.tile([NR, NCAND], F32, tag="uu")
    nc.vector.tensor_scalar_mul(uu[:], E1c, av[0:NR, 0:1])
    up = small.tile([NR, NCAND], F32, tag="up")
    nc.vector.tensor_scalar_mul(up[:], e2[:], avpP[:, 0:1])
    ub = small.tile([NR, NCAND], F32, tag="ub")
    nc.vector.tensor_add(ub[:], uu[:], up[:])
    pm = small.tile([NR, 1], F32, tag="pm")
    nc.vector.reduce_max(pm[:], ub[:], axis=AX.X)
    pswP = pone.tile([NR, 256], F32, tag="po")
    nc.tensor.matmul(pswP[:, 0:1], P64c, pm[:], start=True, stop=True,
                     tile_position=(0, 0))
    sm = small.tile([NR, 1], F32, tag="sm")
    nc.vector.tensor_add(sm[:], pm[:], pswP[:, 0:1])
    df = small.tile([NR, 1], F32, tag="df")
    nc.vector.tensor_sub(df[:], pm[:], pswP[:, 0:1])
    ng = small.tile([NR, 1], F32, tag="ng")
    nc.vector.tensor_scalar_mul(ng[:], df[:], -1.0)
    ad = small.tile([NR, 1], F32, tag="ad")
    nc.vector.tensor_max(ad[:], df[:], ng[:])
    mx = small.tile([NR, 1], F32, tag="mx")
    nc.vector.tensor_add(mx[:], sm[:], ad[:])
    pke = small.tile([NR, 1], F32, tag="pke")  # 0.5*(s+|d|) + eps
    nc.vector.tensor_scalar(pke[:], mx[:], 0.5, EPS, op0=ALU.mult,
                            op1=ALU.add)
    rg = small.tile([NR, 1], F32, tag="rg")
    nc.vector.reciprocal(rg[:], pke[:])
    # transpose rows 0..31 into a free-dim row, then ones-broadcast to [112]
    rgTP = pone.tile([1, 256], F32, tag="po")
    nc.tensor.matmul(rgTP[:, 0:NMAPS], rg[0:NMAPS, 0:1], ID32c, start=True,
                     stop=True, is_transpose=True, tile_position=(0, 0))
    rgT = small.tile([1, NMAPS], F32, tag="rgT")
    nc.vector.tensor_copy(rgT[:], rgTP[:, 0:NMAPS])
    rgBP = pone.tile([P, 512], F32, tag="po")
    nc.tensor.matmul(rgBP[:, 0:NMAPS], onesC, rgT[:], start=True, stop=True,
                     tile_position=(0, 0))
    rgB = const.tile([P, NMAPS], F32)
    nc.vector.tensor_copy(rgB[:], rgBP[:, 0:NMAPS])

    # DRAM view matching stage layout: out[m, y, x], y = 3p+c, z = 336c+x
    dview = out_t.ap().rearrange("m (p c) x -> p m (c x)", p=P)

    def drain(eng, sview, pview, j):
        if eng == "scalar":
            nc.scalar.mul(sview, pview, rgB[:, j:j + 1])
        else:
            nc.vector.tensor_scalar_mul(sview, pview, rgB[:, j:j + 1])

    bacc_ = 0
    groups = [4, 4, 4, 4, 4, 4, 4, 2, 2]
    j0 = 0
    for gi, gsz in enumerate(groups):
        st = sstage.tile([P, 4, NCH * W], BF16, tag="sst")
        for j in range(j0, j0 + gsz):
            q, b = j % 4, j // 4
            rhs = FXT[32 * q:32 * q + 2, b, :]
            lhsT = [FYT[32 * q:32 * q + 2, b, cix::3] for cix in range(NCH)]
            pt = pone.tile([P, 1024], F32, tag="po")
            # chunk c1 is split at the col-512 bank boundary (psum matmul
            # dests must stay inside one 2KB bank)
            nc.tensor.matmul(pt[:, 0:W], lhsT[0], rhs, start=True,
                             stop=True, tile_position=(32 * q, 0))
            nc.tensor.matmul(pt[:, W:512], lhsT[1], rhs[:, 0:512 - W],
                             start=True, stop=True,
                             tile_position=(32 * q, 0))
            nc.tensor.matmul(pt[:, 512:2 * W], lhsT[1], rhs[:, 512 - W:],
                             start=True, stop=True,
                             tile_position=(32 * q, 0))
            nc.tensor.matmul(pt[:, 2 * W:3 * W], lhsT[2], rhs, start=True,
                             stop=True, tile_position=(32 * q, 0))
            stj = st[:, j - j0, :]
            bacc_ += 17  # 17 of 32 map drains on ACT (ACT is faster)
            eng = "scalar" if bacc_ >= 32 else "vector"
            if bacc_ >= 32:
                bacc_ -= 32
            drain(eng, stj, pt[:, 0:NCH * W], j)
        eng = nc.sync if gi % 2 == 0 else nc.gpsimd
        eng.dma_start(dview[:, j0:j0 + gsz, :], st[:, 0:gsz, :])
        j0 += gsz


@functools.lru_cache(maxsize=1)
def _build():
    nc = bacc.Bacc("TRN2", target_bir_lowering=False, debug=False)
    negcd_in = nc.dram_tensor("negcd", [NF, 3 + W + 2 * NCAND], F32,
                              kind="ExternalInput")
    out_t = nc.dram_tensor("out", [NMAPS, H, W], BF16, kind="ExternalOutput")

    aux = np.zeros((NF, NR + NF + NMAPS + P), dtype=np.float32)
    for k in range(NR):  # P64: out[m] = in[(m+32)%64]
        aux[k, (k + NMAPS) % NR] = 1.0
    for k in range(NF):  # P128: out[m] = in[(m+64)%128]
        aux[k, NR + (k + NR) % NF] = 1.0
    aux[0:NMAPS, NR + NF:NR + NF + NMAPS] = np.eye(NMAPS, dtype=np.float32)
    aux[0, NR + NF + NMAPS:] = 1.0
    aux_in = nc.inline_tensor(aux, name="auxc")

    stg = nc.dram_tensor("stg", [2, NR, W], BF16)

    with tile.TileContext(nc) as tc, ExitStack() as ctx:
        _emit(nc, tc, ctx, negcd_in, out_t, aux_in, stg)
    nc.compile()
    return nc


_grid = (np.arange(W, dtype=np.float64) / (W - 1)).astype(np.float32)
_tk = (np.arange(NCAND, dtype=np.float64) * CSTEP).astype(np.float32)
_e1 = np.exp(-(_tk.astype(np.float64) ** 2) / (2.0 * SIGMA ** 2)).astype(
    np.float32)
_ROWC = np.tile(np.concatenate([_grid, _e1, _tk]), (NF, 1))


def _in_map_for(gaze, hand, b):
    cg = np.asarray(gaze[b], dtype=np.float32).reshape(NMAPS, 2)
    ch = np.asarray(hand[b], dtype=np.float32).reshape(NMAPS, 2)
    d = np.sqrt(((cg - ch) ** 2).sum(axis=1))
    cxs = np.concatenate([cg[:, 0], ch[:, 0]])  # t-major per axis
    cys = np.concatenate([cg[:, 1], ch[:, 1]])
    dd = np.concatenate([d, d])
    # rows 0..63: x factors (-cx, -cy, -d); rows 64..127: y (-cy, -cx, -d);
    # then the row-constants [grid | E1 | tk] are packed alongside so one
    # DMA carries every per-row operand
    top = np.stack([-cxs, -cys, -dd], axis=1)
    bot = np.stack([-cys, -cxs, -dd], axis=1)
    ncd = np.concatenate([top, bot], axis=0)
    return {"negcd": np.ascontiguousarray(np.concatenate(
        [ncd, _ROWC], axis=1).astype(np.float32))}


def kernel(gaze_coords, hand_coords, _trace=False, **trace_kwargs):
    gaze_coords = np.asarray(gaze_coords, dtype=np.float32)
    hand_coords = np.asarray(hand_coords, dtype=np.float32)
    B = gaze_coords.shape[0]
    assert B == N_CORES, f"expected batch {N_CORES}, got {B}"
    nc = _build()
    in_maps = [_in_map_for(gaze_coords, hand_coords, b) for b in range(B)]
    res = run_bass_kernel_spmd(nc, in_maps, list(range(N_CORES)),
                               trace=_trace, **trace_kwargs)
    out = np.stack(
        [np.asarray(res.results[i]["out"]).astype(np.float32).reshape(
            S_DIM, C_DIM, H, W) for i in range(B)],
        axis=0,
    )
    if _trace:
        return out, res
    return out
